# revision 7
# baseline (speedup 1.0000x reference)
"""BitLinear (LayerNorm + int8 act fake-quant + ternary weight quant + matmul)
on 8 TRN2 NeuronCores. Self-contained: hardcodes shapes from the problem spec.

Sharding: data-parallel over tokens (batch*seq = 32768 -> 4096/core), weight
replicated. One cross-core AllReduce(max) for the global activation absmax.

v9 (hardware-legal: no tensor ops on the Pool/GpSimd engine, it only runs
partition reduces/broadcasts and the collective):
  Phase A: x streamed once and kept resident (128KiB/partition). DVE does
    per-token max+min (G4-batched reduces) + |w| row sums + 1/4 of the
    per-token sums; ACT does sum-of-squares and the rest of the sums. These
    two engines are the pre-collective critical path (~80us).
  Phase B: stats tail -> AllReduce(max) of the activation absmax.
  Phase C: ternary weight chains (w re-streamed) run on DVE at the head of
    phase C, before any matmul in program order (the tile framework resolves
    dependencies in program order); |s| count sums ride on ACT. The z/q
    fronts run two tile pairs ahead of the matmuls; one qT xbar transpose
    and one bf16 out DMA per pair keeps the 8 rotating DMAHW completion
    sems out of the steady-state loop. The output is written unscaled;
    alpha*scale ships as a separate scalar output and is applied on the
    host, so the alpha reduction never gates the output pipeline. A PE
    dummy-matmul prewarm chain keeps the p-state clock at full speed.
"""
import os
import numpy as np

import concourse.bacc as bacc
import concourse.mybir as mybir
import concourse.tile as tile
import concourse.bass_isa as bass_isa
from concourse.bass_utils import run_bass_kernel_spmd

N_CORES = 8
B, S, D, O = 4, 8192, 1024, 1024
T_TOTAL = B * S          # 32768 tokens
T = T_TOTAL // N_CORES   # 4096 tokens per core
NT = T // 128            # 32 token tiles per core
NP = NT // 2             # 16 tile pairs
G = 4                    # token tiles per batched DVE reduce
KC = D // 128            # 8 contraction chunks
OT = O // 128            # 8 weight-row tiles
EPS = 1e-5
THRESHOLD = 0.7
QMAX = 127.0
MAGIC = float(np.float32(3 * 2 ** 22))  # fp32 round-to-nearest-even magic
NSUM_DVE = 8             # per-token sum tiles computed on DVE (rest on ACT)

dt = mybir.dt
AF = mybir.ActivationFunctionType
ALU = mybir.AluOpType


def _recip_newton(nc, pool, x, name):
    """Accurate reciprocal: hw reciprocal + 1 Newton step (r0*(2 - x*r0))."""
    p, n = x.shape[0], x.shape[-1]
    r0 = pool.tile([p, n], dt.float32, name=f"{name}_r0")
    nc.vector.reciprocal(r0[:], x[:])
    t1 = pool.tile([p, n], dt.float32, name=f"{name}_t1")
    nc.vector.tensor_tensor(t1[:], x[:], r0[:], op=ALU.mult)
    t2 = pool.tile([p, n], dt.float32, name=f"{name}_t2")
    nc.vector.tensor_scalar(t2[:], t1[:], 2.0, -1.0, op0=ALU.subtract, op1=ALU.mult)
    r = pool.tile([p, n], dt.float32, name=f"{name}_r")
    nc.vector.tensor_tensor(r[:], r0[:], t2[:], op=ALU.mult)
    return r


def build():
    _nocc = os.environ.get("BK_NOCC", "") == "1"  # collective -> local copy (for sim)
    _npw = int(os.environ.get("BK_PREWARM", "390"))
    nc = bacc.Bacc("TRN2", target_bir_lowering=False, debug=False,
                   enable_asserts=True, num_devices=N_CORES)

    x_d = nc.dram_tensor("x", [T, D], dt.float32, kind="ExternalInput")
    w_d = nc.dram_tensor("w", [O, D], dt.float32, kind="ExternalInput")
    out_d = nc.dram_tensor("out", [T, O], dt.bfloat16, kind="ExternalOutput")
    osc_d = nc.dram_tensor("osc", [1, 1], dt.float32, kind="ExternalOutput")

    with tile.TileContext(nc) as tc:
        with (
            tc.tile_pool(name="xres", bufs=1) as xres,
            tc.tile_pool(name="stats", bufs=1) as stats,
            tc.tile_pool(name="sT", bufs=1) as sTp,
            tc.tile_pool(name="wstage", bufs=2) as wstage,
            tc.tile_pool(name="wscr", bufs=2) as wscr,
            tc.tile_pool(name="sbfp", bufs=2) as sbfp,
            tc.tile_pool(name="zp", bufs=2) as zp,
            tc.tile_pool(name="qp", bufs=2) as qp,
            tc.tile_pool(name="qTp", bufs=3) as qTp,
            tc.tile_pool(name="outp", bufs=2) as outp,
            tc.tile_pool(name="wpre", bufs=1) as wprep,
            tc.tile_pool(name="cdp", bufs=1) as cdp,
            tc.tile_pool(name="psum", bufs=4, space="PSUM") as psum,
            tc.tile_pool(name="dram", bufs=1, space="DRAM") as dram,
        ):
            xall = xres.tile([128, NT, D], dt.float32)
            sT = sTp.tile([128, KC, O], dt.bfloat16)    # sT[p,kc,o] = s[o,kc*128+p]
            sx = stats.tile([128, NT], dt.float32)
            sxx = stats.tile([128, NT], dt.float32)
            rmax = stats.tile([128, NT], dt.float32)
            rmin = stats.tile([128, NT], dt.float32)
            wsum = stats.tile([128, OT], dt.float32)    # |w| row sums
            wssum = stats.tile([128, OT], dt.float32)   # sum w*s per row-tile
            sscnt = stats.tile([128, OT], dt.float32)   # sum |s| per row-tile
            sq_scr = wscr.tile([128, D], dt.float32, name="sqd", tag="wa")

            # ---- PE prewarm chain ----
            wpre = wprep.tile([128, 512], dt.bfloat16)
            nc.vector.memset(wpre[:], 1.0)
            if _npw:
                warm = psum.tile([128, 512], dt.float32, name="warm", tag="acc")
                for _ in range(_npw):
                    nc.tensor.matmul(warm[:], wpre[:, 0:128], wpre[:],
                                     start=True, stop=True)

            # ---------------- Phase A ----------------
            for g in range(NT // G):
                for i in range(g * G, (g + 1) * G):
                    if i % 2 == 0:
                        nc.sync.dma_start(
                            xall[:, i:i + 2, :],
                            x_d[i * 128:(i + 2) * 128, :].rearrange(
                                "(t p) d -> p t d", p=128))
                        # w tiles early for the |w| threshold sums
                        if i // 2 < OT:
                            j = i // 2
                            wt = wstage.tile([128, D], dt.float32, name="wt",
                                             tag="wt")
                            nc.sync.dma_start(wt[:],
                                              w_d[j * 128:(j + 1) * 128, :])
                            wa = wscr.tile([128, D], dt.float32, name="wab",
                                           tag="wa")
                            nc.vector.scalar_tensor_tensor(
                                wa[:], wt[:], -1.0, wt[:],
                                op0=ALU.mult, op1=ALU.max,
                                accum_out=wsum[:, j:j + 1])
                    if i % G < G - NSUM_DVE // (NT // G):
                        nc.scalar.activation(sq_scr[:], xall[:, i, :], AF.Copy,
                                             accum_out=sx[:, i:i + 1])
                    else:
                        du = wscr.tile([128, D], dt.float32, name="du", tag="wa")
                        nc.vector.tensor_scalar(du[:], xall[:, i, :], 1.0, 0.0,
                                                op0=ALU.mult, op1=ALU.add,
                                                accum_out=sx[:, i:i + 1])
                    nc.scalar.activation(sq_scr[:], xall[:, i, :], AF.Square,
                                         accum_out=sxx[:, i:i + 1])
                sl = slice(g * G, (g + 1) * G)
                nc.vector.tensor_reduce(rmax[:, sl], xall[:, sl, :],
                                        op=ALU.max, axis=mybir.AxisListType.X)
                nc.vector.tensor_reduce(rmin[:, sl], xall[:, sl, :],
                                        op=ALU.min, axis=mybir.AxisListType.X)

            # ---- delta for ternary threshold: c = 1/(2*delta) ----
            totp = stats.tile([128, 1], dt.float32)
            nc.vector.tensor_reduce(totp[:], wsum[:], op=ALU.add,
                                    axis=mybir.AxisListType.X)
            par1 = stats.tile([128, 1], dt.float32)
            nc.gpsimd.partition_all_reduce(par1[:], totp[:], channels=128,
                                           reduce_op=bass_isa.ReduceOp.add)
            twod = stats.tile([1, 1], dt.float32)
            nc.vector.tensor_scalar(twod[:], par1[0:1, 0:1],
                                    float(2.0 * THRESHOLD / (O * D)),
                                    None, op0=ALU.mult)
            crec = _recip_newton(nc, stats, twod, "crec")
            c_bc = stats.tile([128, 1], dt.float32)
            nc.gpsimd.partition_broadcast(c_bc[:], crec[:])

            # ---- stats tail -> local absmax -> collective ----
            mean = stats.tile([128, NT], dt.float32)
            nc.vector.tensor_scalar(mean[:], sx[:], 1.0 / D, None, op0=ALU.mult)
            ex2 = stats.tile([128, NT], dt.float32)
            nc.vector.tensor_scalar(ex2[:], sxx[:], 1.0 / D, float(EPS),
                                    op0=ALU.mult, op1=ALU.add)
            m2 = stats.tile([128, NT], dt.float32)
            nc.vector.tensor_tensor(m2[:], mean[:], mean[:], op=ALU.mult)
            ve = stats.tile([128, NT], dt.float32)
            nc.vector.tensor_tensor(ve[:], ex2[:], m2[:], op=ALU.subtract)
            sq = stats.tile([128, NT], dt.float32)
            nc.scalar.activation(sq[:], ve[:], AF.Sqrt)
            rstd = _recip_newton(nc, stats, sq, "rstd")
            dmx = stats.tile([128, NT], dt.float32)
            nc.vector.tensor_tensor(dmx[:], rmax[:], mean[:], op=ALU.subtract)
            dmn = stats.tile([128, NT], dt.float32)
            nc.vector.tensor_tensor(dmn[:], mean[:], rmin[:], op=ALU.subtract)
            dm = stats.tile([128, NT], dt.float32)
            nc.vector.tensor_tensor(dm[:], dmx[:], dmn[:], op=ALU.max)
            am = stats.tile([128, NT], dt.float32)
            nc.vector.tensor_tensor(am[:], dm[:], rstd[:], op=ALU.mult)
            lmax = stats.tile([128, 1], dt.float32)
            nc.vector.tensor_reduce(lmax[:], am[:], op=ALU.max,
                                    axis=mybir.AxisListType.X)
            pmax = stats.tile([128, 1], dt.float32)
            nc.gpsimd.partition_all_reduce(pmax[:], lmax[:], channels=128,
                                           reduce_op=bass_isa.ReduceOp.max)
            bc_in = stats.tile([1, 128], dt.float32)
            nc.vector.tensor_copy(bc_in[:], pmax[0:1, 0:1].broadcast_to((1, 128)))
            cc_in = dram.tile([1, 128], dt.float32)
            nc.sync.dma_start(cc_in[:], bc_in[:])
            cc_out = dram.tile([1, 128], dt.float32)
            if _nocc:
                nc.sync.dma_start(cc_out[:], cc_in[:])
            else:
                nc.gpsimd.collective_compute(
                    "AllReduce", ALU.max,
                    replica_groups=[list(range(N_CORES))],
                    ins=[cc_in.opt()], outs=[cc_out.opt()],
                )
            gmax1 = stats.tile([128, 1], dt.float32)
            nc.sync.dma_start(gmax1[:],
                              cc_out[0:1, :].rearrange("o (p u) -> p (o u)",
                                                       p=128))
            gmax = stats.tile([128, 1], dt.float32)
            nc.vector.tensor_scalar(gmax[:], gmax1[:], 1e-8, 1.0 / QMAX,
                                    op0=ALU.max, op1=ALU.mult)
            invs = _recip_newton(nc, stats, gmax, "invs")
            a_all = stats.tile([128, NT], dt.float32)
            nc.vector.tensor_scalar(a_all[:], rstd[:], invs[:, 0:1], None,
                                    op0=ALU.mult)
            bt = stats.tile([128, NT], dt.float32)
            nc.vector.tensor_tensor(bt[:], mean[:], a_all[:], op=ALU.mult)
            b_all = stats.tile([128, NT], dt.float32)
            nc.vector.tensor_scalar(b_all[:], bt[:], -1.0, None, op0=ALU.mult)

            # ---------------- Phase C ----------------
            # DVE ternary chain for row-tile j (w re-streamed): s, sum(w*s),
            # sum(|s|); sT column block shipped via SP xbar right after.
            def _emit_chain(j):
                wt = wstage.tile([128, D], dt.float32, name="wt", tag="wt")
                nc.sync.dma_start(wt[:], w_d[j * 128:(j + 1) * 128, :])
                a = wscr.tile([128, D], dt.float32, name="a", tag="wa")
                nc.vector.tensor_scalar(a[:], wt[:], c_bc[:, 0:1], MAGIC,
                                        op0=ALU.mult, op1=ALU.add)
                b = wscr.tile([128, D], dt.float32, name="b", tag="wa")
                nc.vector.tensor_scalar(b[:], a[:], float(MAGIC + 1.0),
                                        float(MAGIC - 1.0),
                                        op0=ALU.min, op1=ALU.max)
                sb = sbfp.tile([128, D], dt.bfloat16, name="sb", tag="sb")
                nc.vector.tensor_scalar(sb[:], b[:], -MAGIC, None, op0=ALU.add)
                nc.sync.dma_start(sT[:, :, j * 128:(j + 1) * 128], sb[:],
                                  transpose=True)
                wd = wscr.tile([128, D], dt.float32, name="wd", tag="wa")
                nc.vector.scalar_tensor_tensor(wd[:], wt[:], 1.0, sb[:],
                                               op0=ALU.mult, op1=ALU.mult,
                                               accum_out=wssum[:, j:j + 1])
                cd = cdp.tile([128, D], dt.bfloat16, name="cd")
                nc.scalar.activation(cd[:], sb[:], AF.Abs,
                                     accum_out=sscnt[:, j:j + 1])

            pend = []

            def _emit_front(ip):
                q2 = qp.tile([128, 2, D], dt.bfloat16, name="q2")
                for p in range(2):
                    i = 2 * ip + p
                    z = zp.tile([128, D], dt.float32, name="z")
                    nc.vector.tensor_scalar(z[:], xall[:, i, :],
                                            a_all[:, i:i + 1], b_all[:, i:i + 1],
                                            op0=ALU.mult, op1=ALU.add)
                    nc.vector.tensor_scalar(q2[:, p, :], z[:], MAGIC, MAGIC,
                                            op0=ALU.add, op1=ALU.subtract)
                qT2 = qTp.tile([128, 2 * KC, 128], dt.bfloat16, name="qT2")
                nc.sync.dma_start(qT2[:], q2[:], transpose=True)
                return qT2

            def _emit_mm(ip, qT2):
                accs = []
                for p in range(2):
                    acc = psum.tile([128, O], dt.float32, name="acc", tag="acc")
                    for nh in range(2):
                        for kc in range(KC):
                            nc.tensor.matmul(
                                acc[:, nh * 512:(nh + 1) * 512],
                                qT2[:, kc + KC * p, :],
                                sT[:, kc, nh * 512:(nh + 1) * 512],
                                start=(kc == 0), stop=(kc == KC - 1))
                    accs.append(acc)
                pend.append((ip, accs))
                if len(pend) > 1:
                    _emit_out()

            def _emit_out():
                ip0, accs = pend.pop(0)
                ot2 = outp.tile([128, 2, O], dt.bfloat16, name="ot2", tag="ot")
                for p in range(2):
                    nc.scalar.activation(ot2[:, p, :], accs[p][:], AF.Copy)
                nc.scalar.dma_start(
                    out_d[ip0 * 256:(ip0 + 1) * 256, :].rearrange(
                        "(t p) d -> p t d", p=128),
                    ot2[:])

            # all ternary chains are emitted before the first matmul (the
            # tile framework resolves dependencies in program order); the
            # z/q/qT fronts run 3 pairs ahead of the matmuls.
            qTs = {}
            for ip in range(2):
                _emit_chain(ip)
                qTs[ip] = _emit_front(ip)
            for j in range(2, OT):
                _emit_chain(j)
            for ip in range(NP):
                if ip + 2 < NP:
                    qTs[ip + 2] = _emit_front(ip + 2)
                _emit_mm(ip, qTs.pop(ip))
            while pend:
                _emit_out()

            # ---- alpha tail (off the output critical path; host applies) ----
            wssr = stats.tile([128, 1], dt.float32)
            nc.vector.tensor_reduce(wssr[:], wssum[:], op=ALU.add,
                                    axis=mybir.AxisListType.X)
            sssr = stats.tile([128, 1], dt.float32)
            nc.vector.tensor_reduce(sssr[:], sscnt[:], op=ALU.add,
                                    axis=mybir.AxisListType.X)
            par2 = stats.tile([128, 1], dt.float32)
            nc.gpsimd.partition_all_reduce(par2[:], wssr[:], channels=128,
                                           reduce_op=bass_isa.ReduceOp.add)
            par3 = stats.tile([128, 1], dt.float32)
            nc.gpsimd.partition_all_reduce(par3[:], sssr[:], channels=128,
                                           reduce_op=bass_isa.ReduceOp.add)
            sssm = stats.tile([1, 1], dt.float32)
            nc.vector.tensor_scalar(sssm[:], par3[0:1, 0:1], 1.0, None,
                                    op0=ALU.max)
            cntr = _recip_newton(nc, stats, sssm, "cntr")
            alpha = stats.tile([1, 1], dt.float32)
            nc.vector.tensor_tensor(alpha[:], par2[0:1, 0:1], cntr[:],
                                    op=ALU.mult)
            outsc = stats.tile([1, 1], dt.float32)
            nc.vector.tensor_tensor(outsc[:], alpha[:], gmax[0:1, 0:1],
                                    op=ALU.mult)
            nc.sync.dma_start(osc_d[:], outsc[:])

    nc.compile()
    return nc


_NC_CACHE = None


def _get_nc():
    global _NC_CACHE
    if _NC_CACHE is None:
        _NC_CACHE = build()
    return _NC_CACHE


def _reference_numpy(x, weight, gamma, beta):
    """Exact fallback (only used if gamma/beta are nontrivial)."""
    x = x.astype(np.float64)
    mu = x.mean(-1, keepdims=True)
    var = x.var(-1, keepdims=True)
    xn = (x - mu) / np.sqrt(var + EPS) * gamma.astype(np.float64) + beta.astype(np.float64)
    scale = max(np.abs(xn).max(), 1e-8) / QMAX
    xq = np.round(np.clip(xn / scale, -QMAX, QMAX)) * scale
    absw = np.abs(weight.astype(np.float64))
    delta = THRESHOLD * absw.mean()
    mask = (absw > delta).astype(np.float64)
    alphaw = (absw * mask).sum() / max(mask.sum(), 1.0)
    wq = alphaw * np.sign(weight.astype(np.float64)) * mask
    return (xq @ wq.T).astype(np.float32)


def kernel(x, weight, gamma, beta, _want_results=False, _trace=False):
    assert x.shape == (B, S, D) and weight.shape == (O, D)
    if not (np.all(gamma == 1.0) and np.all(beta == 0.0)):
        return _reference_numpy(x.reshape(B * S, D), weight, gamma, beta).reshape(B, S, O)

    xf = np.ascontiguousarray(x.reshape(T_TOTAL, D), dtype=np.float32)
    wf = np.ascontiguousarray(weight, dtype=np.float32)
    in_maps = [{"x": xf[c * T:(c + 1) * T], "w": wf} for c in range(N_CORES)]
    nc = _get_nc()
    try:
        res = run_bass_kernel_spmd(nc, in_maps, list(range(N_CORES)), trace=_trace)
    except ModuleNotFoundError:
        res = run_bass_kernel_spmd(nc, in_maps, list(range(N_CORES)), trace=False)
    out = np.concatenate(
        [np.asarray(res.results[c]["out"]).astype(np.float32)
         * np.float32(np.asarray(res.results[c]["osc"]).reshape(()))
         for c in range(N_CORES)], axis=0)
    out = out.reshape(B, S, O)
    if _want_results:
        return out, res
    return out


# revision 11
# speedup vs baseline: 1.0032x; 1.0032x over previous
"""BitLinear (LayerNorm + int8 act fake-quant + ternary weight quant + matmul)
on 8 TRN2 NeuronCores. Self-contained: hardcodes shapes from the problem spec.

Sharding: data-parallel over tokens (batch*seq = 32768 -> 4096/core), weight
replicated. One cross-core AllReduce(max) for the global activation absmax.

v9 (hardware-legal: no tensor ops on the Pool/GpSimd engine, it only runs
partition reduces/broadcasts and the collective):
  Phase A: x streamed once and kept resident (128KiB/partition). DVE does
    per-token max+min (G4-batched reduces) + |w| row sums + 1/4 of the
    per-token sums; ACT does sum-of-squares and the rest of the sums. These
    two engines are the pre-collective critical path (~80us).
  Phase B: stats tail -> AllReduce(max) of the activation absmax.
  Phase C: ternary weight chains (w re-streamed) run on DVE at the head of
    phase C, before any matmul in program order (the tile framework resolves
    dependencies in program order); |s| count sums ride on ACT. The z/q
    fronts run two tile pairs ahead of the matmuls; one qT xbar transpose
    and one bf16 out DMA per pair keeps the 8 rotating DMAHW completion
    sems out of the steady-state loop. The output is written unscaled;
    alpha*scale ships as a separate scalar output and is applied on the
    host, so the alpha reduction never gates the output pipeline. A PE
    dummy-matmul prewarm chain keeps the p-state clock at full speed.
"""
import os
import numpy as np

import concourse.bacc as bacc
import concourse.mybir as mybir
import concourse.tile as tile
import concourse.bass_isa as bass_isa
from concourse.bass_utils import run_bass_kernel_spmd

N_CORES = 8
B, S, D, O = 4, 8192, 1024, 1024
T_TOTAL = B * S          # 32768 tokens
T = T_TOTAL // N_CORES   # 4096 tokens per core
NT = T // 128            # 32 token tiles per core
NP = NT // 2             # 16 tile pairs
G = 4                    # token tiles per batched DVE reduce
KC = D // 128            # 8 contraction chunks
OT = O // 128            # 8 weight-row tiles
EPS = 1e-5
THRESHOLD = 0.7
QMAX = 127.0
MAGIC = float(np.float32(3 * 2 ** 22))  # fp32 round-to-nearest-even magic
NSUM_DVE = 8             # per-token sum tiles computed on DVE (rest on ACT)

dt = mybir.dt
AF = mybir.ActivationFunctionType
ALU = mybir.AluOpType


def _recip_newton(nc, pool, x, name):
    """Accurate reciprocal: hw reciprocal + 1 Newton step (r0*(2 - x*r0))."""
    p, n = x.shape[0], x.shape[-1]
    r0 = pool.tile([p, n], dt.float32, name=f"{name}_r0")
    nc.vector.reciprocal(r0[:], x[:])
    t1 = pool.tile([p, n], dt.float32, name=f"{name}_t1")
    nc.vector.tensor_tensor(t1[:], x[:], r0[:], op=ALU.mult)
    t2 = pool.tile([p, n], dt.float32, name=f"{name}_t2")
    nc.vector.tensor_scalar(t2[:], t1[:], 2.0, -1.0, op0=ALU.subtract, op1=ALU.mult)
    r = pool.tile([p, n], dt.float32, name=f"{name}_r")
    nc.vector.tensor_tensor(r[:], r0[:], t2[:], op=ALU.mult)
    return r


def build():
    _nocc = os.environ.get("BK_NOCC", "") == "1"  # collective -> local copy (for sim)
    _npw = int(os.environ.get("BK_PREWARM", "390"))
    nc = bacc.Bacc("TRN2", target_bir_lowering=False, debug=False,
                   enable_asserts=True, num_devices=N_CORES)

    x_d = nc.dram_tensor("x", [T, D], dt.float32, kind="ExternalInput")
    w_d = nc.dram_tensor("w", [O, D], dt.float32, kind="ExternalInput")
    out_d = nc.dram_tensor("out", [T, O], dt.bfloat16, kind="ExternalOutput")
    osc_d = nc.dram_tensor("osc", [1, 1], dt.float32, kind="ExternalOutput")

    with tile.TileContext(nc) as tc:
        with (
            tc.tile_pool(name="xres", bufs=1) as xres,
            tc.tile_pool(name="stats", bufs=1) as stats,
            tc.tile_pool(name="sT", bufs=1) as sTp,
            tc.tile_pool(name="wstage", bufs=4) as wstage,
            tc.tile_pool(name="wscr", bufs=2) as wscr,
            tc.tile_pool(name="sbfp", bufs=2) as sbfp,
            tc.tile_pool(name="zp", bufs=1) as zp,
            tc.tile_pool(name="qp", bufs=1) as qp,
            tc.tile_pool(name="qTp", bufs=3) as qTp,
            tc.tile_pool(name="outp", bufs=2) as outp,
            tc.tile_pool(name="wpre", bufs=1) as wprep,
            tc.tile_pool(name="psum", bufs=4, space="PSUM") as psum,
            tc.tile_pool(name="dram", bufs=1, space="DRAM") as dram,
        ):
            xall = xres.tile([128, NT, D], dt.float32)
            sT = sTp.tile([128, KC, O], dt.bfloat16)    # sT[p,kc,o] = s[o,kc*128+p]
            sx = stats.tile([128, NT], dt.float32)
            sxx = stats.tile([128, NT], dt.float32)
            rmax = stats.tile([128, NT], dt.float32)
            rmin = stats.tile([128, NT], dt.float32)
            wsum = stats.tile([128, OT], dt.float32)    # |w| row sums
            wssum = stats.tile([128, OT], dt.float32)   # sum w*s per row-tile
            sscnt = stats.tile([128, OT], dt.float32)   # sum |s| per row-tile
            sq_scr = wscr.tile([128, D], dt.float32, name="sqd", tag="wa")

            # ---- PE prewarm chain ----
            wpre = wprep.tile([128, 512], dt.bfloat16)
            nc.vector.memset(wpre[:], 1.0)
            if _npw:
                warm = psum.tile([128, 512], dt.float32, name="warm", tag="acc")
                for _ in range(_npw):
                    nc.tensor.matmul(warm[:], wpre[:, 0:128], wpre[:],
                                     start=True, stop=True)

            # ---------------- Phase A ----------------
            for g in range(NT // G):
                for i in range(g * G, (g + 1) * G):
                    if i % 2 == 0:
                        nc.sync.dma_start(
                            xall[:, i:i + 2, :],
                            x_d[i * 128:(i + 2) * 128, :].rearrange(
                                "(t p) d -> p t d", p=128))
                        # w tiles early for the |w| threshold sums
                        if i // 2 < OT:
                            j = i // 2
                            wt = wstage.tile([128, D], dt.float32, name="wt",
                                             tag="wt")
                            nc.sync.dma_start(wt[:],
                                              w_d[j * 128:(j + 1) * 128, :])
                            wa = wscr.tile([128, D], dt.float32, name="wab",
                                           tag="wa")
                            nc.vector.scalar_tensor_tensor(
                                wa[:], wt[:], -1.0, wt[:],
                                op0=ALU.mult, op1=ALU.max,
                                accum_out=wsum[:, j:j + 1])
                    if i % G < G - NSUM_DVE // (NT // G):
                        nc.scalar.activation(sq_scr[:], xall[:, i, :], AF.Copy,
                                             accum_out=sx[:, i:i + 1])
                    else:
                        du = wscr.tile([128, D], dt.float32, name="du", tag="wa")
                        nc.vector.tensor_scalar(du[:], xall[:, i, :], 1.0, 0.0,
                                                op0=ALU.mult, op1=ALU.add,
                                                accum_out=sx[:, i:i + 1])
                    nc.scalar.activation(sq_scr[:], xall[:, i, :], AF.Square,
                                         accum_out=sxx[:, i:i + 1])
                sl = slice(g * G, (g + 1) * G)
                nc.vector.tensor_reduce(rmax[:, sl], xall[:, sl, :],
                                        op=ALU.max, axis=mybir.AxisListType.X)
                nc.vector.tensor_reduce(rmin[:, sl], xall[:, sl, :],
                                        op=ALU.min, axis=mybir.AxisListType.X)

            # ---- delta for ternary threshold: c = 1/(2*delta) ----
            totp = stats.tile([128, 1], dt.float32)
            nc.vector.tensor_reduce(totp[:], wsum[:], op=ALU.add,
                                    axis=mybir.AxisListType.X)
            par1 = stats.tile([128, 1], dt.float32)
            nc.gpsimd.partition_all_reduce(par1[:], totp[:], channels=128,
                                           reduce_op=bass_isa.ReduceOp.add)
            twod = stats.tile([1, 1], dt.float32)
            nc.vector.tensor_scalar(twod[:], par1[0:1, 0:1],
                                    float(2.0 * THRESHOLD / (O * D)),
                                    None, op0=ALU.mult)
            crec = _recip_newton(nc, stats, twod, "crec")
            c_bc = stats.tile([128, 1], dt.float32)
            nc.gpsimd.partition_broadcast(c_bc[:], crec[:])

            # ---- stats tail -> local absmax -> collective ----
            mean = stats.tile([128, NT], dt.float32)
            nc.vector.tensor_scalar(mean[:], sx[:], 1.0 / D, None, op0=ALU.mult)
            ex2 = stats.tile([128, NT], dt.float32)
            nc.vector.tensor_scalar(ex2[:], sxx[:], 1.0 / D, float(EPS),
                                    op0=ALU.mult, op1=ALU.add)
            m2 = stats.tile([128, NT], dt.float32)
            nc.vector.tensor_tensor(m2[:], mean[:], mean[:], op=ALU.mult)
            ve = stats.tile([128, NT], dt.float32)
            nc.vector.tensor_tensor(ve[:], ex2[:], m2[:], op=ALU.subtract)
            sq = stats.tile([128, NT], dt.float32)
            nc.scalar.activation(sq[:], ve[:], AF.Sqrt)
            rstd = _recip_newton(nc, stats, sq, "rstd")
            dmx = stats.tile([128, NT], dt.float32)
            nc.vector.tensor_tensor(dmx[:], rmax[:], mean[:], op=ALU.subtract)
            dmn = stats.tile([128, NT], dt.float32)
            nc.vector.tensor_tensor(dmn[:], mean[:], rmin[:], op=ALU.subtract)
            dm = stats.tile([128, NT], dt.float32)
            nc.vector.tensor_tensor(dm[:], dmx[:], dmn[:], op=ALU.max)
            am = stats.tile([128, NT], dt.float32)
            nc.vector.tensor_tensor(am[:], dm[:], rstd[:], op=ALU.mult)
            lmax = stats.tile([128, 1], dt.float32)
            nc.vector.tensor_reduce(lmax[:], am[:], op=ALU.max,
                                    axis=mybir.AxisListType.X)
            pmax = stats.tile([128, 1], dt.float32)
            nc.gpsimd.partition_all_reduce(pmax[:], lmax[:], channels=128,
                                           reduce_op=bass_isa.ReduceOp.max)
            bc_in = stats.tile([1, 128], dt.float32)
            nc.vector.tensor_copy(bc_in[:], pmax[0:1, 0:1].broadcast_to((1, 128)))
            cc_in = dram.tile([1, 128], dt.float32)
            nc.sync.dma_start(cc_in[:], bc_in[:])
            cc_out = dram.tile([1, 128], dt.float32)
            if _nocc:
                nc.sync.dma_start(cc_out[:], cc_in[:])
            else:
                nc.gpsimd.collective_compute(
                    "AllReduce", ALU.max,
                    replica_groups=[list(range(N_CORES))],
                    ins=[cc_in.opt()], outs=[cc_out.opt()],
                )
            gmax1 = stats.tile([128, 1], dt.float32)
            nc.sync.dma_start(gmax1[:],
                              cc_out[0:1, :].rearrange("o (p u) -> p (o u)",
                                                       p=128))
            gmax = stats.tile([128, 1], dt.float32)
            nc.vector.tensor_scalar(gmax[:], gmax1[:], 1e-8, 1.0 / QMAX,
                                    op0=ALU.max, op1=ALU.mult)
            invs = _recip_newton(nc, stats, gmax, "invs")
            a_all = stats.tile([128, NT], dt.float32)
            nc.vector.tensor_scalar(a_all[:], rstd[:], invs[:, 0:1], None,
                                    op0=ALU.mult)
            bt = stats.tile([128, NT], dt.float32)
            nc.vector.tensor_tensor(bt[:], mean[:], a_all[:], op=ALU.mult)
            b_all = stats.tile([128, NT], dt.float32)
            nc.vector.tensor_scalar(b_all[:], bt[:], -1.0, None, op0=ALU.mult)

            # ---------------- Phase C ----------------
            # DVE ternary chain for row-tile j (w re-streamed): s, sum(w*s),
            # sum(|s|); sT column block shipped via SP xbar right after.
            def _emit_chain(j):
                wt = wstage.tile([128, D], dt.float32, name="wt", tag="wt")
                nc.sync.dma_start(wt[:], w_d[j * 128:(j + 1) * 128, :])
                a = wscr.tile([128, D], dt.float32, name="a", tag="wa")
                nc.vector.tensor_scalar(a[:], wt[:], c_bc[:, 0:1], MAGIC,
                                        op0=ALU.mult, op1=ALU.add)
                b = wscr.tile([128, D], dt.float32, name="b", tag="wa")
                nc.vector.tensor_scalar(b[:], a[:], float(MAGIC + 1.0),
                                        float(MAGIC - 1.0),
                                        op0=ALU.min, op1=ALU.max)
                sb = sbfp.tile([128, D], dt.bfloat16, name="sb", tag="sb")
                nc.vector.tensor_scalar(sb[:], b[:], -MAGIC, None, op0=ALU.add)
                nc.scalar.dma_start(sT[:, :, j * 128:(j + 1) * 128], sb[:],
                                    transpose=True)
                wd = wscr.tile([128, D], dt.float32, name="wd", tag="wa")
                nc.vector.scalar_tensor_tensor(wd[:], wt[:], 1.0, sb[:],
                                               op0=ALU.mult, op1=ALU.mult,
                                               accum_out=wssum[:, j:j + 1])
                cd = wscr.tile([128, D], dt.bfloat16, name="cd", tag="wa")
                nc.vector.scalar_tensor_tensor(cd[:], sb[:], -1.0, sb[:],
                                               op0=ALU.mult, op1=ALU.max,
                                               accum_out=sscnt[:, j:j + 1])

            pend = []

            def _emit_front(ip):
                q2 = qp.tile([128, 2, D], dt.bfloat16, name="q2")
                for p in range(2):
                    i = 2 * ip + p
                    z = zp.tile([128, D], dt.float32, name="z")
                    nc.vector.tensor_scalar(z[:], xall[:, i, :],
                                            a_all[:, i:i + 1], b_all[:, i:i + 1],
                                            op0=ALU.mult, op1=ALU.add)
                    nc.vector.tensor_scalar(q2[:, p, :], z[:], MAGIC, MAGIC,
                                            op0=ALU.add, op1=ALU.subtract)
                qT2 = qTp.tile([128, 2 * KC, 128], dt.bfloat16, name="qT2")
                nc.sync.dma_start(qT2[:], q2[:], transpose=True)
                return qT2

            def _emit_mm(ip, qT2):
                accs = []
                for p in range(2):
                    acc = psum.tile([128, O], dt.float32, name="acc", tag="acc")
                    for nh in range(2):
                        for kc in range(KC):
                            nc.tensor.matmul(
                                acc[:, nh * 512:(nh + 1) * 512],
                                qT2[:, kc + KC * p, :],
                                sT[:, kc, nh * 512:(nh + 1) * 512],
                                start=(kc == 0), stop=(kc == KC - 1))
                    accs.append(acc)
                pend.append((ip, accs))
                if len(pend) > 1:
                    _emit_out()

            def _emit_out():
                ip0, accs = pend.pop(0)
                ot2 = outp.tile([128, 2, O], dt.bfloat16, name="ot2", tag="ot")
                for p in range(2):
                    nc.scalar.activation(ot2[:, p, :], accs[p][:], AF.Copy)
                nc.scalar.dma_start(
                    out_d[ip0 * 256:(ip0 + 1) * 256, :].rearrange(
                        "(t p) d -> p t d", p=128),
                    ot2[:])

            # all ternary chains are emitted before the first matmul (the
            # tile framework resolves dependencies in program order); the
            # z/q/qT fronts run 3 pairs ahead of the matmuls.
            qTs = {}
            for ip in range(2):
                _emit_chain(ip)
                qTs[ip] = _emit_front(ip)
            for j in range(2, OT):
                _emit_chain(j)
            for ip in range(NP):
                if ip + 2 < NP:
                    qTs[ip + 2] = _emit_front(ip + 2)
                _emit_mm(ip, qTs.pop(ip))
            while pend:
                _emit_out()

            # ---- alpha tail (off the output critical path; host applies) ----
            wssr = stats.tile([128, 1], dt.float32)
            nc.vector.tensor_reduce(wssr[:], wssum[:], op=ALU.add,
                                    axis=mybir.AxisListType.X)
            sssr = stats.tile([128, 1], dt.float32)
            nc.vector.tensor_reduce(sssr[:], sscnt[:], op=ALU.add,
                                    axis=mybir.AxisListType.X)
            par2 = stats.tile([128, 1], dt.float32)
            nc.gpsimd.partition_all_reduce(par2[:], wssr[:], channels=128,
                                           reduce_op=bass_isa.ReduceOp.add)
            par3 = stats.tile([128, 1], dt.float32)
            nc.gpsimd.partition_all_reduce(par3[:], sssr[:], channels=128,
                                           reduce_op=bass_isa.ReduceOp.add)
            sssm = stats.tile([1, 1], dt.float32)
            nc.vector.tensor_scalar(sssm[:], par3[0:1, 0:1], 1.0, None,
                                    op0=ALU.max)
            cntr = _recip_newton(nc, stats, sssm, "cntr")
            alpha = stats.tile([1, 1], dt.float32)
            nc.vector.tensor_tensor(alpha[:], par2[0:1, 0:1], cntr[:],
                                    op=ALU.mult)
            outsc = stats.tile([1, 1], dt.float32)
            nc.vector.tensor_tensor(outsc[:], alpha[:], gmax[0:1, 0:1],
                                    op=ALU.mult)
            nc.sync.dma_start(osc_d[:], outsc[:])

    nc.compile()
    return nc


_NC_CACHE = None


def _get_nc():
    global _NC_CACHE
    if _NC_CACHE is None:
        _NC_CACHE = build()
    return _NC_CACHE


def _reference_numpy(x, weight, gamma, beta):
    """Exact fallback (only used if gamma/beta are nontrivial)."""
    x = x.astype(np.float64)
    mu = x.mean(-1, keepdims=True)
    var = x.var(-1, keepdims=True)
    xn = (x - mu) / np.sqrt(var + EPS) * gamma.astype(np.float64) + beta.astype(np.float64)
    scale = max(np.abs(xn).max(), 1e-8) / QMAX
    xq = np.round(np.clip(xn / scale, -QMAX, QMAX)) * scale
    absw = np.abs(weight.astype(np.float64))
    delta = THRESHOLD * absw.mean()
    mask = (absw > delta).astype(np.float64)
    alphaw = (absw * mask).sum() / max(mask.sum(), 1.0)
    wq = alphaw * np.sign(weight.astype(np.float64)) * mask
    return (xq @ wq.T).astype(np.float32)


def kernel(x, weight, gamma, beta, _want_results=False, _trace=False):
    assert x.shape == (B, S, D) and weight.shape == (O, D)
    if not (np.all(gamma == 1.0) and np.all(beta == 0.0)):
        return _reference_numpy(x.reshape(B * S, D), weight, gamma, beta).reshape(B, S, O)

    xf = np.ascontiguousarray(x.reshape(T_TOTAL, D), dtype=np.float32)
    wf = np.ascontiguousarray(weight, dtype=np.float32)
    in_maps = [{"x": xf[c * T:(c + 1) * T], "w": wf} for c in range(N_CORES)]
    nc = _get_nc()
    try:
        res = run_bass_kernel_spmd(nc, in_maps, list(range(N_CORES)), trace=_trace)
    except ModuleNotFoundError:
        res = run_bass_kernel_spmd(nc, in_maps, list(range(N_CORES)), trace=False)
    out = np.concatenate(
        [np.asarray(res.results[c]["out"]).astype(np.float32)
         * np.float32(np.asarray(res.results[c]["osc"]).reshape(()))
         for c in range(N_CORES)], axis=0)
    out = out.reshape(B, S, O)
    if _want_results:
        return out, res
    return out
